# revision 26
# baseline (speedup 1.0000x reference)
"""Trainium2 Bass kernel for a single-head cross-attention block.

Reference computation (per batch b of B=128):
    q = input[b] @ Wq            # [T,H]   T=512, C=384, H=64
    k = x[b] @ Wk                # [T,H]
    v = x[b] @ Wv                # [T,H]
    S = (q @ k.T) * C**-0.5      # [T,T], causal mask
    P = softmax(S, axis=-1)
    out[b] = P @ v               # [T,H]

Strategy: data-parallel over 8 NeuronCores (16 batches each).

v2 highlights over the 86us baseline:
  - q and k projections are COL-TILED: Wq occupies PE columns 0:63 and
    Wk columns 64:127, so the two M=64 projections stream their rhs
    concurrently -> one [128, 512] PSUM tile holds [qT; kT] in half the
    PE time.
  - One [128,512] DVE cast PSUM->SBUF (qk_sb) + a partition-swapped
    copy kq_sb = [kT; qT] built on the idle GpSimd engine. With both
    layouts, the K=64 S^T matmuls are ROW-TILED two-at-a-time (rows
    0:63 and 64:127 of the PE run independent matmuls).
  - S^T chunks packed in one [128, 1280] PSUM tile arranged so each
    concurrent pair writes different PSUM banks; exp runs as 2 scalar
    activations ([0,1024) and [1024,1280)).
  - Causal masking via tri-mask multiply on GpSimd; softmax denominator
    via a ones-column in v (shipped to host, which divides).
  - Output drained to bf16 by DVE and DMA'd from the vector queue;
    inputs arrive as 2-batch interleaved DMAs on the sync queue.
"""

import numpy as np
import ml_dtypes

import concourse.bass as bass
import concourse.tile as tile
import concourse.mybir as mybir
from concourse.bass_utils import run_bass_kernel_spmd
from concourse.masks import make_upper_triangular

N_CORES = 8
B, T, C, H = 128, 512, 384, 64
BPC = B // N_CORES          # batches per core
CK = C // 128               # contraction chunks for projections
TK = T // 128               # T chunks
SCALE = float(C) ** -0.5
BF16 = mybir.dt.bfloat16
F32 = mybir.dt.float32
EXP = mybir.ActivationFunctionType.Exp

_bf16 = ml_dtypes.bfloat16

# Packed layout of causal S^T chunks inside one [128, 1280] PSUM tile.
# Row-tiled pairs write different banks (bank = 512 f32):
#   bank0: m0rest @0 (384 wide), diag0 @384
#   bank1: diag1 @512, diag3 @640, m1rest @768 (256 wide)
#   bank2: diag2 @1024, m2rest @1152 (128 wide)
DIAG_BASE = {0: 384, 1: 512, 2: 1024, 3: 640}
REST_BASE = {0: 0, 1: 768, 2: 1152}
ST_PACKED = 1280
# e2 holds the masked diagonal blocks in the order diag0,diag1,diag3,diag2
# (matches one contiguous mul over e[384:768] plus one over e[1024:1152])
E2_POS = {0: 0, 1: 128, 2: 384, 3: 256}


def _split_multi_waits(nc: bass.Bass):
    """walrus in this build encodes at most ONE sync-wait per instruction.
    Tile's wait-assignment can attach several. Move the extras onto
    same-engine NOPs inserted immediately before each instruction —
    identical semantics (the engine blocks on the NOP waits first)."""
    n = 0
    for bb in nc.m.functions[0].blocks:
        new_insts = []
        for inst in bb.instructions:
            si = inst.sync_info
            waits = list(si.on_wait) if si and si.on_wait else []
            if len(waits) > 1:
                for w in waits[:-1]:
                    nop = mybir.InstNoOp(name=f"WSPLIT-{n}", ins=[], outs=[])
                    n += 1
                    nop.engine = inst.engine
                    nop.sync_info = mybir.SyncInfo(on_wait=[w], on_update=[])
                    new_insts.append(nop)
                si.on_wait = waits[-1:]
            new_insts.append(inst)
        bb.instructions[:] = new_insts
    return nc


NPAIR = BPC // 2


def build_kernel() -> bass.Bass:
    nc = bass.Bass()
    # two batches interleaved per DMA: [pair, 128, 2(batch), 2(in/x), CK, T]
    inx = nc.dram_tensor("inx", [NPAIR, 128, 2, 2, CK, T], BF16,
                         kind="ExternalInput")
    wall = nc.dram_tensor("wall", [128, 3, CK, H], BF16, kind="ExternalInput")
    out = nc.dram_tensor("out", [BPC, 128, TK, H + 1], BF16,
                         kind="ExternalOutput")

    PREFETCH = 2  # pairs

    with tile.TileContext(nc) as tc:
        with (
            tc.tile_pool(name="const", bufs=1) as const_pool,
            tc.tile_pool(name="inputs", bufs=PREFETCH + 1) as in_pool,
            tc.tile_pool(name="qk", bufs=2) as qk_pool,
            tc.tile_pool(name="e", bufs=2) as e_pool,
            tc.tile_pool(name="osb", bufs=4) as o_pool,
            tc.tile_pool(name="qk_ps", bufs=1, space="PSUM") as qk_psum,
            tc.tile_pool(name="st_ps", bufs=1, space="PSUM") as st_psum,
            tc.tile_pool(name="v_ps", bufs=2, space="PSUM") as v_psum,
            tc.tile_pool(name="o_ps", bufs=2, space="PSUM") as o_psum,
        ):
            w_sb = const_pool.tile([128, 3, CK, H], BF16, tag="wall")
            nc.scalar.dma_start(w_sb[:], wall[:])
            # 4x-tiled upper-triangular (incl diagonal) 0/1 causal mask
            tri4 = const_pool.tile([128, 4 * 128], BF16, tag="tri4")
            for j in range(TK):
                make_upper_triangular(nc, tri4[:, 128 * j:128 * (j + 1)],
                                      val=1.0, diag=True)
            # two persistent v tiles (ones column written once each)
            v_tiles = [const_pool.tile([128, TK, H + 1], BF16, tag=f"v{i}",
                                       name=f"v{i}")
                       for i in range(2)]
            for vt in v_tiles:
                nc.gpsimd.memset(vt[:, :, H], 1.0)

            in_tiles = {}
            state = {}

            def emit_load(p, split=False):
                it = in_pool.tile([128, 2, 2, CK, T], BF16, tag="inx")
                if split:
                    nc.sync.dma_start(it[:, 0], inx[p, :, 0])
                    nc.sync.dma_start(it[:, 1], inx[p, :, 1])
                else:
                    nc.sync.dma_start(it[:], inx[p])
                in_tiles[p] = it

            def emit_qkv(b):
                pair = in_tiles[b // 2]
                it = pair[:, b % 2, 0]
                xt = pair[:, b % 2, 1]
                if b % 2 == 1:
                    del in_tiles[b // 2]
                # col-tiled q (cols 0:64) / k (cols 64:128) projections
                qk_ps = qk_psum.tile([128, TK, 128], F32, tag="qk")
                qk_sb = qk_pool.tile([128, T], BF16, tag="qk_sb")
                klo_sb = qk_pool.tile([64, 2, 128], BF16, tag="klo_sb")
                qhi_sb = qk_pool.tile([128, 384], BF16, tag="qhi_sb")
                for c in range(CK):
                    nc.tensor.matmul(
                        qk_ps[0:64], w_sb[:, 0, c, :], it[:, c, :],
                        start=(c == 0), stop=(c == CK - 1),
                    )
                    nc.tensor.matmul(
                        qk_ps[64:128], w_sb[:, 1, c, :], xt[:, c, :],
                        start=(c == 0), stop=(c == CK - 1),
                    )
                nc.vector.tensor_copy(qk_sb[:], qk_ps[:])
                # minimal relocated slices for row-tiled S^T:
                #   klo = kT chunks {0,2} on partitions 0:64 (row0 stationary)
                #   qhi1/qhi2 = qT cols 128:256 / 256:512 on partitions
                #   64:128 (row64 rhs), split DVE/scalar by need time
                nc.vector.tensor_copy(klo_sb[:], qk_ps[64:128, 0::2, :])
                nc.scalar.copy(qhi_sb[64:128, :], qk_ps[0:64, 1:4, :])

                v_ps = v_psum.tile([128, TK, H], F32, tag="v")
                for t in range(TK):
                    for c in range(CK):
                        nc.tensor.matmul(
                            v_ps[:, t, :],
                            xt[:, c, 128 * t:128 * (t + 1)],
                            w_sb[:, 2, c, :],
                            start=(c == 0), stop=(c == CK - 1),
                        )
                v_sb = v_tiles[b % 2]
                nc.vector.tensor_copy(v_sb[:, :, 0:H], v_ps[:])
                state[b] = [qk_sb, klo_sb, qhi_sb, v_sb]

            def emit_st(b):
                qk_sb, klo_sb, qhi_sb, _ = state[b]
                # one PSUM tile per bank so Tile's (tile-granular) dependency
                # tracking never serializes one bank's consumers against
                # another bank's producers:
                #   stA: m0rest @0:384,  diag0 @384:512
                #   stB: diag1 @0:128,   diag3 @128:256,  m1rest @256:512
                #   stC: diag2 @0:128,   m2rest @128:256
                st = st_psum.tile([128, 1408], F32, tag="st")
                eD = e_pool.tile([128, 512], BF16, tag="eD", name="eD")
                eM0 = e_pool.tile([128, 384], BF16, tag="eM0", name="eM0")
                eM12 = e_pool.tile([128, 384], BF16, tag="eM12", name="eM12")

                def mm(dst, ncols, lhsT, rhs):
                    nc.tensor.matmul(st[:, dst:dst + ncols], lhsT, rhs,
                                     start=True, stop=True)

                def kT_hi(m):   # kT chunk m on partitions 64:128 (native)
                    return qk_sb[64:128, 128 * m:128 * (m + 1)]

                def q_hi(lo, hi):  # qT cols (qhi holds qT[128:512] @64:128)
                    return qhi_sb[64:128, lo - 128:hi - 128]

                # Packing: bankA m0rest@0:384 (pad to 512); bankB all four
                # diag blocks @512+128t; bankC m1rest@1024, m2rest@1280.
                # Concurrent PE row streams (640 cycles each), paired so no
                # two concurrent writes share a PSUM bank, and each bank's
                # pc-last writer also physically finishes last:
                #   row0  = m0rest, diag0 (stationary kT0=klo[0]), diag2 (klo[1])
                #   row64 = diag1, diag3, m1rest, m2rest (stationary native kT)
                mm(512 + 128, 128, kT_hi(1), q_hi(128, 256))           # diag1
                mm(512 + 384, 128, kT_hi(3), q_hi(384, 512))           # diag3
                mm(0, 384, klo_sb[:, 0], qk_sb[0:64, 128:512])         # m0rest
                mm(1024, 256, kT_hi(1), q_hi(256, 512))                # m1rest
                mm(512, 128, klo_sb[:, 0], qk_sb[0:64, 0:128])         # diag0
                mm(1280, 128, kT_hi(2), q_hi(384, 512))                # m2rest
                mm(512 + 256, 128, klo_sb[:, 1], qk_sb[0:64, 256:384]) # diag2
                # diag bank exp'd FIRST so one DVE mul unblocks every PV diag
                nc.scalar.activation(eD[:], st[:, 512:1024], EXP, scale=SCALE)
                nc.scalar.activation(eM0[:], st[:, 0:384], EXP, scale=SCALE)
                nc.scalar.activation(eM12[:], st[:, 1024:1408], EXP,
                                     scale=SCALE)
                state[b].append((eD, eM0, eM12))

            def emit_masks(b):
                eD = state[b][4][0]
                e2 = e_pool.tile([128, 512], BF16, tag="e2")
                nc.vector.tensor_mul(e2[:], eD[:], tri4[:])
                state[b].append(e2)

            def emit_pv(b, split_store=False):
                _, _, _, v_sb, (eD, eM0, eM12), e2 = state.pop(b)
                o_ps = o_psum.tile([128, TK, H + 1], F32, tag="o")
                o_sb = o_pool.tile([128, TK, H + 1], BF16, tag="o_sb")
                # (t, m) sources: m==t masked diag from e2 @128t; m=0 from
                # eM0; m=1 from eM12 @0:256; m=2 from eM12 @256:384.
                def src(t, m):
                    if m == t:
                        return e2[:, 128 * t:128 * (t + 1)]
                    if m == 0:
                        return eM0[:, 128 * (t - 1):128 * t]
                    if m == 1:
                        return eM12[:, 128 * (t - 2):128 * (t - 1)]
                    return eM12[:, 256:384]

                def pv_mm(t, m):
                    nc.tensor.matmul(o_ps[:, t, :], src(t, m), v_sb[:, m, :],
                                     start=(m == 0), stop=(m == t))

                if split_store:
                    for t in range(TK):
                        for m in range(t + 1):
                            pv_mm(t, m)
                        # drain each query chunk as soon as its chain closes
                        nc.vector.tensor_copy(o_sb[:, t, :], o_ps[:, t, :])
                        nc.sync.dma_start(out[b, :, t], o_sb[:, t, :])
                else:
                    # groups stay contiguous (start=True clears has_written
                    # bank-wide: interleaved groups in one bank corrupt
                    # accumulation)
                    for t in (0, 1, 2, 3):
                        for m in range(t + 1):
                            pv_mm(t, m)
                    nc.vector.tensor_copy(o_sb[:], o_ps[:])
                    nc.sync.dma_start(out[b], o_sb[:])

            for p in range(min(PREFETCH, NPAIR)):
                emit_load(p, split=(p == 0))
            for b in range(BPC):
                if b % 2 == 0 and b // 2 + PREFETCH < NPAIR:
                    emit_load(b // 2 + PREFETCH)
                emit_qkv(b)
                if b > 0:
                    emit_masks(b - 1)
                    emit_pv(b - 1)
                emit_st(b)
            emit_masks(BPC - 1)
            emit_pv(BPC - 1, split_store=True)
    _split_multi_waits(nc)
    return nc


def _layout_input(a: np.ndarray) -> np.ndarray:
    """[n, T, C] f32 -> [n, 128, CK, T] bf16, partition-major."""
    a = np.asarray(a, dtype=np.float32)
    n = a.shape[0]
    a = a.transpose(0, 2, 1).reshape(n, CK, 128, T).transpose(0, 2, 1, 3)
    return np.ascontiguousarray(a).astype(_bf16)


def _layout_weights(Wq, Wk, Wv) -> np.ndarray:
    """three [C, H] -> [128, 3, CK, H] bf16."""
    def lay(w):
        w = np.asarray(w, dtype=np.float32)
        return w.reshape(CK, 128, H).transpose(1, 0, 2)
    return np.ascontiguousarray(
        np.stack([lay(Wq), lay(Wk), lay(Wv)], axis=1)).astype(_bf16)


def prepare_in_maps(input, x, Wq, Wk, Wv):
    inpT = _layout_input(input)
    xT = _layout_input(x)
    # interleave per (batch, partition): [B, 128, 2, CK, T]
    inx = np.stack([inpT, xT], axis=2)
    # pair consecutive batches: [B//2, 2, 128, 2, CK, T] -> [B//2, 128, 2, 2, CK, T]
    inx = inx.reshape(B // 2, 2, 128, 2, CK, T).transpose(0, 2, 1, 3, 4, 5)
    inx = np.ascontiguousarray(inx)
    wall = _layout_weights(Wq, Wk, Wv)
    in_maps = []
    for c in range(N_CORES):
        sl = slice(c * NPAIR, (c + 1) * NPAIR)
        in_maps.append({
            "inx": np.ascontiguousarray(inx[sl]),
            "wall": wall,
        })
    return in_maps


def postprocess(results) -> np.ndarray:
    # device layout [BPC, 128, TK, H+1]: [b, p, t, 0:H] is the unnormalized
    # PV sum for query row 128t+p; [..., H] is the softmax denominator.
    outs = []
    for r in results:
        raw = r["out"].reshape(BPC, 128, TK, H + 1).astype(np.float32)
        o = raw[..., :H] / raw[..., H:]
        outs.append(o.transpose(0, 2, 1, 3).reshape(BPC, T, H))
    return np.concatenate(outs, axis=0).astype(np.float32)


_cached_nc = None


def kernel(input: np.ndarray, x: np.ndarray, Wq: np.ndarray, Wk: np.ndarray,
           Wv: np.ndarray) -> np.ndarray:
    global _cached_nc
    if _cached_nc is None:
        _cached_nc = build_kernel()
    nc = _cached_nc
    in_maps = prepare_in_maps(input, x, Wq, Wk, Wv)
    res = run_bass_kernel_spmd(nc, in_maps, core_ids=list(range(N_CORES)))
    return postprocess(res.results)


# revision 31
# speedup vs baseline: 1.0258x; 1.0258x over previous
"""Trainium2 Bass kernel for a single-head cross-attention block.

Reference computation (per batch b of B=128):
    q = input[b] @ Wq            # [T,H]   T=512, C=384, H=64
    k = x[b] @ Wk                # [T,H]
    v = x[b] @ Wv                # [T,H]
    S = (q @ k.T) * C**-0.5      # [T,T], causal mask
    P = softmax(S, axis=-1)
    out[b] = P @ v               # [T,H]

Strategy: data-parallel over 8 NeuronCores (16 batches each).

v2 highlights over the 86us baseline:
  - q and k projections are COL-TILED: Wq occupies PE columns 0:63 and
    Wk columns 64:127, so the two M=64 projections stream their rhs
    concurrently -> one [128, 512] PSUM tile holds [qT; kT] in half the
    PE time.
  - One [128,512] DVE cast PSUM->SBUF (qk_sb) + a partition-swapped
    copy kq_sb = [kT; qT] built on the idle GpSimd engine. With both
    layouts, the K=64 S^T matmuls are ROW-TILED two-at-a-time (rows
    0:63 and 64:127 of the PE run independent matmuls).
  - S^T chunks packed in one [128, 1280] PSUM tile arranged so each
    concurrent pair writes different PSUM banks; exp runs as 2 scalar
    activations ([0,1024) and [1024,1280)).
  - Causal masking via tri-mask multiply on GpSimd; softmax denominator
    via a ones-column in v (shipped to host, which divides).
  - Output drained to bf16 by DVE and DMA'd from the vector queue;
    inputs arrive as 2-batch interleaved DMAs on the sync queue.
"""

import numpy as np
import ml_dtypes

import concourse.bass as bass
import concourse.tile as tile
import concourse.mybir as mybir
from concourse.bass_utils import run_bass_kernel_spmd
from concourse.masks import make_upper_triangular

N_CORES = 8
B, T, C, H = 128, 512, 384, 64
BPC = B // N_CORES          # batches per core
CK = C // 128               # contraction chunks for projections
TK = T // 128               # T chunks
SCALE = float(C) ** -0.5
BF16 = mybir.dt.bfloat16
F32 = mybir.dt.float32
EXP = mybir.ActivationFunctionType.Exp

_bf16 = ml_dtypes.bfloat16

# Packed layout of causal S^T chunks inside one [128, 1280] PSUM tile.
# Row-tiled pairs write different banks (bank = 512 f32):
#   bank0: m0rest @0 (384 wide), diag0 @384
#   bank1: diag1 @512, diag3 @640, m1rest @768 (256 wide)
#   bank2: diag2 @1024, m2rest @1152 (128 wide)
DIAG_BASE = {0: 384, 1: 512, 2: 1024, 3: 640}
REST_BASE = {0: 0, 1: 768, 2: 1152}
ST_PACKED = 1280
# e2 holds the masked diagonal blocks in the order diag0,diag1,diag3,diag2
# (matches one contiguous mul over e[384:768] plus one over e[1024:1152])
E2_POS = {0: 0, 1: 128, 2: 384, 3: 256}


def _split_multi_waits(nc: bass.Bass):
    """walrus in this build encodes at most ONE sync-wait per instruction.
    Tile's wait-assignment can attach several. Move the extras onto
    same-engine NOPs inserted immediately before each instruction —
    identical semantics (the engine blocks on the NOP waits first)."""
    n = 0
    for bb in nc.m.functions[0].blocks:
        new_insts = []
        for inst in bb.instructions:
            si = inst.sync_info
            waits = list(si.on_wait) if si and si.on_wait else []
            if len(waits) > 1:
                for w in waits[:-1]:
                    nop = mybir.InstNoOp(name=f"WSPLIT-{n}", ins=[], outs=[])
                    n += 1
                    nop.engine = inst.engine
                    nop.sync_info = mybir.SyncInfo(on_wait=[w], on_update=[])
                    new_insts.append(nop)
                si.on_wait = waits[-1:]
            new_insts.append(inst)
        bb.instructions[:] = new_insts
    return nc


NPAIR = BPC // 2


def build_kernel() -> bass.Bass:
    nc = bass.Bass()
    # two batches interleaved per DMA: [pair, 128, 2(batch), 2(in/x), CK, T]
    inx = nc.dram_tensor("inx", [NPAIR, 128, 2, 2, CK, T], BF16,
                         kind="ExternalInput")
    wall = nc.dram_tensor("wall", [128, 3, CK, H], BF16, kind="ExternalInput")
    out = nc.dram_tensor("out", [BPC, 128, TK, H + 1], BF16,
                         kind="ExternalOutput")

    PREFETCH = 2  # pairs

    with tile.TileContext(nc) as tc:
        with (
            tc.tile_pool(name="const", bufs=1) as const_pool,
            tc.tile_pool(name="inputs", bufs=PREFETCH + 1) as in_pool,
            tc.tile_pool(name="qk", bufs=2) as qk_pool,
            tc.tile_pool(name="e", bufs=2) as e_pool,
            tc.tile_pool(name="osb", bufs=4) as o_pool,
            tc.tile_pool(name="qk_ps", bufs=1, space="PSUM") as qk_psum,
            tc.tile_pool(name="st_ps", bufs=1, space="PSUM") as st_psum,
            tc.tile_pool(name="v_ps", bufs=2, space="PSUM") as v_psum,
            tc.tile_pool(name="o_ps", bufs=2, space="PSUM") as o_psum,
        ):
            w_sb = const_pool.tile([128, 3, CK, H], BF16, tag="wall")
            nc.scalar.dma_start(w_sb[:], wall[:])
            # 4x-tiled upper-triangular (incl diagonal) 0/1 causal mask
            tri4 = const_pool.tile([128, 4 * 128], BF16, tag="tri4")
            for j in range(TK):
                make_upper_triangular(nc, tri4[:, 128 * j:128 * (j + 1)],
                                      val=1.0, diag=True)
            # two persistent v tiles (ones column written once each)
            v_tiles = [const_pool.tile([128, TK, H + 1], BF16, tag=f"v{i}",
                                       name=f"v{i}")
                       for i in range(2)]
            for vt in v_tiles:
                nc.gpsimd.memset(vt[:, :, H], 1.0)

            in_tiles = {}
            state = {}

            def emit_load(p, split=False):
                it = in_pool.tile([128, 2, 2, CK, T], BF16, tag="inx")
                if split:
                    nc.sync.dma_start(it[:, 0], inx[p, :, 0])
                    nc.sync.dma_start(it[:, 1], inx[p, :, 1])
                else:
                    nc.sync.dma_start(it[:], inx[p])
                in_tiles[p] = it

            def emit_qkv(b):
                pair = in_tiles[b // 2]
                it = pair[:, b % 2, 0]
                xt = pair[:, b % 2, 1]
                if b % 2 == 1:
                    del in_tiles[b // 2]
                # col-tiled q (cols 0:64) / k (cols 64:128) projections
                qk_ps = qk_psum.tile([128, TK, 128], F32, tag="qk")
                qk_sb = qk_pool.tile([128, T], BF16, tag="qk_sb")
                klo_sb = qk_pool.tile([64, 2, 128], BF16, tag="klo_sb")
                qhi_sb = qk_pool.tile([128, 384], BF16, tag="qhi_sb")
                for c in range(CK):
                    nc.tensor.matmul(
                        qk_ps[0:64], w_sb[:, 0, c, :], it[:, c, :],
                        start=(c == 0), stop=(c == CK - 1),
                    )
                    nc.tensor.matmul(
                        qk_ps[64:128], w_sb[:, 1, c, :], xt[:, c, :],
                        start=(c == 0), stop=(c == CK - 1),
                    )
                nc.vector.tensor_copy(qk_sb[:], qk_ps[:])
                # minimal relocated slices for row-tiled S^T:
                #   klo = kT chunks {0,2} on partitions 0:64 (row0 stationary)
                #   qhi1/qhi2 = qT cols 128:256 / 256:512 on partitions
                #   64:128 (row64 rhs), split DVE/scalar by need time
                nc.vector.tensor_copy(klo_sb[:], qk_ps[64:128, 0::2, :])
                nc.scalar.copy(qhi_sb[64:128, :], qk_ps[0:64, 1:4, :])

                v_ps = v_psum.tile([128, TK, H], F32, tag="v")
                for t in range(TK):
                    for c in range(CK):
                        nc.tensor.matmul(
                            v_ps[:, t, :],
                            xt[:, c, 128 * t:128 * (t + 1)],
                            w_sb[:, 2, c, :],
                            start=(c == 0), stop=(c == CK - 1),
                        )
                v_sb = v_tiles[b % 2]
                nc.vector.tensor_copy(v_sb[:, :, 0:H], v_ps[:])
                state[b] = [qk_sb, klo_sb, qhi_sb, v_sb]

            def emit_st(b):
                qk_sb, klo_sb, qhi_sb, _ = state[b]
                # one PSUM tile per bank so Tile's (tile-granular) dependency
                # tracking never serializes one bank's consumers against
                # another bank's producers:
                #   stA: m0rest @0:384,  diag0 @384:512
                #   stB: diag1 @0:128,   diag3 @128:256,  m1rest @256:512
                #   stC: diag2 @0:128,   m2rest @128:256
                st = st_psum.tile([128, 1408], F32, tag="st")
                eD = e_pool.tile([128, 512], BF16, tag="eD", name="eD")
                eM0 = e_pool.tile([128, 384], BF16, tag="eM0", name="eM0")
                eM12 = e_pool.tile([128, 384], BF16, tag="eM12", name="eM12")

                def mm(dst, ncols, lhsT, rhs):
                    nc.tensor.matmul(st[:, dst:dst + ncols], lhsT, rhs,
                                     start=True, stop=True)

                def kT_hi(m):   # kT chunk m on partitions 64:128 (native)
                    return qk_sb[64:128, 128 * m:128 * (m + 1)]

                def q_hi(lo, hi):  # qT cols (qhi holds qT[128:512] @64:128)
                    return qhi_sb[64:128, lo - 128:hi - 128]

                # Packing: bankA m0rest@0:384 (pad to 512); bankB all four
                # diag blocks @512+128t; bankC m1rest@1024, m2rest@1280.
                # Concurrent PE row streams (640 cycles each), paired so no
                # two concurrent writes share a PSUM bank, and each bank's
                # pc-last writer also physically finishes last:
                #   row0  = m0rest, diag0 (stationary kT0=klo[0]), diag2 (klo[1])
                #   row64 = diag1, diag3, m1rest, m2rest (stationary native kT)
                mm(512 + 128, 128, kT_hi(1), q_hi(128, 256))           # diag1
                mm(512 + 384, 128, kT_hi(3), q_hi(384, 512))           # diag3
                mm(0, 384, klo_sb[:, 0], qk_sb[0:64, 128:512])         # m0rest
                mm(1024, 256, kT_hi(1), q_hi(256, 512))                # m1rest
                mm(512, 128, klo_sb[:, 0], qk_sb[0:64, 0:128])         # diag0
                mm(1280, 128, kT_hi(2), q_hi(384, 512))                # m2rest
                mm(512 + 256, 128, klo_sb[:, 1], qk_sb[0:64, 256:384]) # diag2
                # diag bank exp'd FIRST so one DVE mul unblocks every PV diag
                nc.scalar.activation(eD[:], st[:, 512:1024], EXP, scale=SCALE)
                nc.scalar.activation(eM0[:], st[:, 0:384], EXP, scale=SCALE)
                nc.scalar.activation(eM12[:], st[:, 1024:1408], EXP,
                                     scale=SCALE)
                state[b].append((eD, eM0, eM12))

            def emit_masks(b):
                eD = state[b][4][0]
                e2a = e_pool.tile([128, 256], BF16, tag="e2a")
                e2b = e_pool.tile([128, 256], BF16, tag="e2b")
                nc.vector.tensor_mul(e2a[:], eD[:, 0:256], tri4[:, 0:256])
                nc.vector.tensor_mul(e2b[:], eD[:, 256:512], tri4[:, 0:256])
                state[b].append((e2a, e2b))

            def emit_pv(b, split_store=False):
                _, _, _, v_sb, (eD, eM0, eM12), (e2a, e2b) = state.pop(b)
                o_ps = o_psum.tile([128, TK, H + 1], F32, tag="o")
                o_sb = o_pool.tile([128, TK, H + 1], BF16, tag="o_sb")
                # (t, m) sources: m==t masked diag from e2 @128t; m=0 from
                # eM0; m=1 from eM12 @0:256; m=2 from eM12 @256:384.
                def src(t, m):
                    if m == t:
                        e2 = e2a if t < 2 else e2b
                        return e2[:, 128 * (t % 2):128 * (t % 2 + 1)]
                    if m == 0:
                        return eM0[:, 128 * (t - 1):128 * t]
                    if m == 1:
                        return eM12[:, 128 * (t - 2):128 * (t - 1)]
                    return eM12[:, 256:384]

                def pv_mm(t, m):
                    nc.tensor.matmul(o_ps[:, t, :], src(t, m), v_sb[:, m, :],
                                     start=(m == 0), stop=(m == t))

                if split_store:
                    for t in range(TK):
                        for m in range(t + 1):
                            pv_mm(t, m)
                        # drain each query chunk as soon as its chain closes
                        nc.vector.tensor_copy(o_sb[:, t, :], o_ps[:, t, :])
                        nc.sync.dma_start(out[b, :, t], o_sb[:, t, :])
                else:
                    # groups stay contiguous (start=True clears has_written
                    # bank-wide: interleaved groups in one bank corrupt
                    # accumulation)
                    for t in (0, 1, 2, 3):
                        for m in range(t + 1):
                            pv_mm(t, m)
                    nc.vector.tensor_copy(o_sb[:], o_ps[:])
                    nc.sync.dma_start(out[b], o_sb[:])

            for p in range(min(PREFETCH, NPAIR)):
                emit_load(p, split=(p == 0))
            for b in range(BPC):
                if b % 2 == 0 and b // 2 + PREFETCH < NPAIR:
                    emit_load(b // 2 + PREFETCH)
                emit_qkv(b)
                if b > 0:
                    emit_masks(b - 1)
                    emit_pv(b - 1)
                emit_st(b)
            emit_masks(BPC - 1)
            emit_pv(BPC - 1, split_store=True)
    _split_multi_waits(nc)
    return nc


def _layout_input(a: np.ndarray) -> np.ndarray:
    """[n, T, C] f32 -> [n, 128, CK, T] bf16, partition-major."""
    a = np.asarray(a, dtype=np.float32)
    n = a.shape[0]
    a = a.transpose(0, 2, 1).reshape(n, CK, 128, T).transpose(0, 2, 1, 3)
    return np.ascontiguousarray(a).astype(_bf16)


def _layout_weights(Wq, Wk, Wv) -> np.ndarray:
    """three [C, H] -> [128, 3, CK, H] bf16."""
    def lay(w):
        w = np.asarray(w, dtype=np.float32)
        return w.reshape(CK, 128, H).transpose(1, 0, 2)
    return np.ascontiguousarray(
        np.stack([lay(Wq), lay(Wk), lay(Wv)], axis=1)).astype(_bf16)


def prepare_in_maps(input, x, Wq, Wk, Wv):
    inpT = _layout_input(input)
    xT = _layout_input(x)
    # interleave per (batch, partition): [B, 128, 2, CK, T]
    inx = np.stack([inpT, xT], axis=2)
    # pair consecutive batches: [B//2, 2, 128, 2, CK, T] -> [B//2, 128, 2, 2, CK, T]
    inx = inx.reshape(B // 2, 2, 128, 2, CK, T).transpose(0, 2, 1, 3, 4, 5)
    inx = np.ascontiguousarray(inx)
    wall = _layout_weights(Wq, Wk, Wv)
    in_maps = []
    for c in range(N_CORES):
        sl = slice(c * NPAIR, (c + 1) * NPAIR)
        in_maps.append({
            "inx": np.ascontiguousarray(inx[sl]),
            "wall": wall,
        })
    return in_maps


def postprocess(results) -> np.ndarray:
    # device layout [BPC, 128, TK, H+1]: [b, p, t, 0:H] is the unnormalized
    # PV sum for query row 128t+p; [..., H] is the softmax denominator.
    outs = []
    for r in results:
        raw = r["out"].reshape(BPC, 128, TK, H + 1).astype(np.float32)
        o = raw[..., :H] / raw[..., H:]
        outs.append(o.transpose(0, 2, 1, 3).reshape(BPC, T, H))
    return np.concatenate(outs, axis=0).astype(np.float32)


_cached_nc = None


def kernel(input: np.ndarray, x: np.ndarray, Wq: np.ndarray, Wk: np.ndarray,
           Wv: np.ndarray) -> np.ndarray:
    global _cached_nc
    if _cached_nc is None:
        _cached_nc = build_kernel()
    nc = _cached_nc
    in_maps = prepare_in_maps(input, x, Wq, Wk, Wv)
    res = run_bass_kernel_spmd(nc, in_maps, core_ids=list(range(N_CORES)))
    return postprocess(res.results)
